# revision 44
# baseline (speedup 1.0000x reference)
"""TRN2 Bass kernel: 3-layer MLP (LN->Linear->GELU)x3, *sqrt(1024).

bf16 datapath, row-major activation tiles [128 rows, D free]. All three
activation transposes go through the DMA XBAR (dma_start transpose=True,
batched 4-16 tiles per dispatch) straight into SBUF, so PE is a pure
matmul stream (bf16, 1 cyc/row, fp32 PSUM accumulate). L0 is packed 4
tiles per pass via block-diagonal weights. LN stats via per-tile DVE
bn_stats + fp32 merge; LN applies on DVE (fp32 math, bf16 out). GELU on
ScalarE from PSUM. Final x32 on DVE (GPSIMD tensor ops are ~40x slower
than modeled on HW - keep it off the datapath). Output stored bf16 (host
converts to fp32; rel-err budget allows it) halving store traffic.
Groups of 16 tiles are software-pipelined two deep: LN0 stats + z0 prep
(stages A+B) of group g+1 and the LN2 apply + T2 XBAR preps of group g
are emitted in iteration g, while the big L2 matmul quads of group g-1
(prepped last iteration) interleave between L0/L1 work of group g - so
PE opens every iteration with its inputs already staged and runs
back-to-back. The x load is prefetched two groups ahead; output stores
ride the GPSIMD SWDGE queue so their semaphore waits never block the SP
HWDGE dispatch queue. 8 cores data-parallel over rows.
"""
import math
import numpy as np
from contextlib import ExitStack

N_CORES = 8
N_ROWS = 262144
F_IN = 6
D1, D2, D3 = 128, 512, 1024
ROWS_PER_CORE = N_ROWS // N_CORES
P = 128
EPS = 1e-5
OUT_SCALE = math.sqrt(1024.0)
MAGIC = 0x5F3759DF
KERNEL_G = 16
B2 = 4  # tiles per batched z2 DMA-transpose
PIPELINE_D = True  # interleave stage D of group g-1 into stages A-C of g

_cache = {}


def _rsqrt_newton(nc, mybir, dt, pool, vp, g, iters=2):
    """y = 1/sqrt(vp), vp fp32 [128, g] positive. Returns y tile."""
    A = mybir.AluOpType
    ti = pool.tile([P, g], dt.int32, name="nt_i")
    nc.vector.tensor_scalar(
        out=ti[:], in0=vp[:].bitcast(dt.int32), scalar1=1, scalar2=-1,
        op0=A.logical_shift_right, op1=A.bitwise_xor)
    y = pool.tile([P, g], dt.float32, name="nt_y")
    nc.vector.tensor_scalar(
        out=y[:].bitcast(dt.int32), in0=ti[:], scalar1=MAGIC + 1, scalar2=None,
        op0=A.add)
    t = pool.tile([P, g], dt.float32, name="nt_t")
    for _ in range(iters):
        nc.vector.tensor_tensor(out=t[:], in0=y[:], in1=y[:], op=A.mult)
        nc.vector.tensor_tensor(out=t[:], in0=t[:], in1=vp[:], op=A.mult)
        nc.vector.tensor_scalar(out=t[:], in0=t[:], scalar1=-0.5, scalar2=1.5,
                                op0=A.mult, op1=A.add)
        nc.vector.tensor_tensor(out=y[:], in0=y[:], in1=t[:], op=A.mult)
    return y


def _ln_finish(nc, mybir, dt, pool, mv6, G, tag, invD):
    """mv6 [128,G,6] = raw bn_stats [n1,m1,v1,n2,m2,v2] per tile; merge the
    two halves: mu=(m1+m2)/2, var=(M2_1+M2_2)/D+((m1-m2)/2)^2. Returns
    (s=1/sqrt(var+eps), c=mu*s, mu)."""
    A = mybir.AluOpType
    m1, v1 = mv6[:, :, 1], mv6[:, :, 2]
    m2, v2 = mv6[:, :, 4], mv6[:, :, 5]
    mu = pool.tile([P, G], dt.float32, name=f"mu{tag}")
    nc.vector.tensor_tensor(out=mu[:], in0=m1, in1=m2, op=A.add)
    dm = pool.tile([P, G], dt.float32, name=f"dm{tag}")
    nc.vector.tensor_tensor(out=dm[:], in0=m1, in1=m2, op=A.subtract)
    nc.vector.tensor_tensor(out=dm[:], in0=dm[:], in1=dm[:], op=A.mult)
    vp = pool.tile([P, G], dt.float32, name=f"vp{tag}")
    nc.vector.tensor_tensor(out=vp[:], in0=v1, in1=v2, op=A.add)
    # vp = (v1+v2)*invD + dm*0.25 + eps
    nc.vector.tensor_scalar(out=dm[:], in0=dm[:], scalar1=0.25, scalar2=EPS,
                            op0=A.mult, op1=A.add)
    nc.vector.tensor_scalar(out=vp[:], in0=vp[:], scalar1=invD, scalar2=None,
                            op0=A.mult)
    nc.vector.tensor_tensor(out=vp[:], in0=vp[:], in1=dm[:], op=A.add)
    s = _rsqrt_newton(nc, mybir, dt, pool, vp, G)
    c = pool.tile([P, G], dt.float32, name=f"c{tag}")
    nc.vector.tensor_scalar(out=mu[:], in0=mu[:], scalar1=0.5, scalar2=None,
                            op0=A.mult)
    nc.vector.tensor_tensor(out=c[:], in0=mu[:], in1=s[:], op=A.mult)
    return s, c, mu, vp


def _build(nc, tile_mod, rows, G, aug0, aug1, aug2, gelu_fn=None):
    from concourse import mybir
    dt = mybir.dt
    A = mybir.AluOpType
    AF = mybir.ActivationFunctionType
    GELU = AF.Gelu if gelu_fn is None else gelu_fn
    ntiles = rows // P
    assert ntiles % G == 0 and G % 4 == 0 and G % B2 == 0

    x_d = nc.dram_tensor("x", [rows, F_IN], dt.float32, kind="ExternalInput")
    w0_d = nc.dram_tensor("w0blk", [P, 4 * D1], dt.bfloat16,
                          kind="ExternalInput")
    w1_d = nc.dram_tensor("w1t", [D1, D2], dt.bfloat16, kind="ExternalInput")
    w2_d = nc.dram_tensor("w2t", [D2, D3], dt.bfloat16, kind="ExternalInput")
    b1_d = nc.dram_tensor("b1aug", [2, D2], dt.bfloat16, kind="ExternalInput")
    b2_d = nc.dram_tensor("b2aug", [2, D3], dt.bfloat16, kind="ExternalInput")
    o_d = nc.dram_tensor("out", [rows, D3], dt.bfloat16, kind="ExternalOutput")

    with tile_mod.TileContext(nc) as tc, ExitStack() as ctx:
        const = ctx.enter_context(tc.tile_pool(name="const", bufs=1))
        xin = ctx.enter_context(tc.tile_pool(name="xin", bufs=3))
        zap = ctx.enter_context(tc.tile_pool(name="zap", bufs=3))
        h1p = ctx.enter_context(tc.tile_pool(name="h1p", bufs=G // 4 + 2))
        h2p = ctx.enter_context(tc.tile_pool(name="h2p", bufs=28))
        sb_b = ctx.enter_context(tc.tile_pool(name="sb_b", bufs=6))
        sb_c = ctx.enter_context(tc.tile_pool(name="sb_c", bufs=4))
        stp = ctx.enter_context(tc.tile_pool(name="stp", bufs=4))
        outp = ctx.enter_context(tc.tile_pool(name="outp", bufs=3))
        ps_b = ctx.enter_context(
            tc.tile_pool(name="ps_b", bufs=4, space="PSUM"))

        w0_sb = const.tile([P, 4 * D1], dt.bfloat16)
        nc.sync.dma_start(w0_sb[:], w0_d[:, :])
        w1_sb = const.tile([D1, D2], dt.bfloat16)
        nc.sync.dma_start(w1_sb[:], w1_d[:, :])
        w2_sb = const.tile([P, 4, D3], dt.bfloat16)
        nc.sync.dma_start(w2_sb[:], w2_d[:, :].rearrange("(k p) o -> p k o", p=P))
        if aug1:
            b1_sb = const.tile([2, D2], dt.bfloat16)
            nc.sync.dma_start(b1_sb[:], b1_d[:, :])
            ones1 = const.tile([2, P], dt.bfloat16)
            nc.vector.memset(ones1[:1, :], 1.0)
            nc.vector.memset(ones1[1:2, :], 0.0)
        if aug2:
            b2_sb = const.tile([2, D3], dt.bfloat16)
            nc.sync.dma_start(b2_sb[:], b2_d[:, :])
            ones2 = const.tile([2, P], dt.bfloat16)
            nc.vector.memset(ones2[:1, :], 1.0)
            nc.vector.memset(ones2[1:2, :], 0.0)

        ngroups = ntiles // G
        pend = {}
        state = {}

        def load_x(g):
            xg = xin.tile([P, G, 8], dt.float32, name="xg")
            nc.sync.dma_start(
                xg[:, :, 0:F_IN],
                x_d[g * G * P:(g + 1) * G * P, :]
                .rearrange("(a p) f -> p a f", p=P))
            pend[g] = xg

        def stage_A(g):
            xg = pend[g]
            mv0 = stp.tile([P, G, 8], dt.float32, name="mv0")
            for gg in range(G):
                nc.vector.bn_stats(out=mv0[:, gg, 0:6], in_=xg[:, gg, 0:F_IN])
            state[g] = _ln_finish(nc, mybir, dt, stp, mv0, G, "0",
                                  1.0 / F_IN)[:2]

        def stage_B(g):
            xg = pend.pop(g)
            s0, c0 = state.pop(g)
            zag = zap.tile([P, 4, 4, 32], dt.bfloat16, name="zag")
            nc.vector.memset(zag[:], 0.0)
            for q in range(G // 4):
                for i in range(4):
                    gg = q * 4 + i
                    nc.vector.tensor_scalar(
                        out=zag[:, q, i, 0:F_IN], in0=xg[:, gg, 0:F_IN],
                        scalar1=s0[:, gg:gg + 1], scalar2=c0[:, gg:gg + 1],
                        op0=A.mult, op1=A.subtract)
            if aug0:
                nc.vector.memset(
                    zag[:, :, :, 6:7].rearrange("p a b c -> p (a b c)"), 1.0)
            z0T = zap.tile([P, 4, P], dt.bfloat16, name="z0T")
            nc.sync.dma_start(z0T[:],
                              zag[:].rearrange("p a b c -> p (a b c)"),
                              transpose=True)
            state[("z0T", g)] = z0T

        def stage_L0(g):
            z0T = state.pop(("z0T", g))
            h1pk = []
            mv1 = stp.tile([P, G, 8], dt.float32, name="mv1")
            for q in range(G // 4):
                u0 = ps_b.tile([P, 4, D1], dt.float32, name="u0", tag="psB",
                               bufs=2)
                nc.tensor.matmul(u0[:].rearrange("p a b -> p (a b)"),
                                 z0T[:, q, :], w0_sb[:], start=True, stop=True)
                h1 = h1p.tile([P, 4, D1], dt.bfloat16, name="h1")
                nc.scalar.activation(
                    out=h1[:].rearrange("p a b -> p (a b)"),
                    in_=u0[:].rearrange("p a b -> p (a b)"), func=GELU)
                h1pk.append(h1)
                for i in range(4):
                    gg = q * 4 + i
                    nc.vector.bn_stats(out=mv1[:, gg, 0:6],
                                       in_=h1[:, i, :])
            s1, c1 = _ln_finish(nc, mybir, dt, stp, mv1, G, "1", 1.0 / D1)[:2]
            return h1pk, s1, c1

        def stage_C_quad(q0, h1pk, s1, c1, h2g, mv2):
            z1b = sb_b.tile([P, 4, D1], dt.bfloat16, name="z1b")
            for t in range(4):
                gg = q0 + t
                nc.vector.tensor_scalar(
                    out=z1b[:, t, :], in0=h1pk[gg // 4][:, gg % 4, :],
                    scalar1=s1[:, gg:gg + 1], scalar2=c1[:, gg:gg + 1],
                    op0=A.mult, op1=A.subtract)
            z1T = sb_b.tile([P, 4, P], dt.bfloat16, name="z1T")
            nc.sync.dma_start(z1T[:],
                              z1b[:].rearrange("p a b -> p (a b)"),
                              transpose=True)
            for t in range(4):
                gg = q0 + t
                u1 = ps_b.tile([P, D2], dt.float32, name="u1", tag="psC",
                               bufs=2)
                nc.tensor.matmul(u1[:], z1T[:, t, :], w1_sb[:],
                                 start=True, stop=not aug1)
                if aug1:
                    nc.tensor.matmul(u1[:], ones1[:], b1_sb[:],
                                     start=False, stop=True)
                h2 = h2p.tile([P, D2], dt.bfloat16, name="h2")
                nc.scalar.activation(out=h2[:], in_=u1[:], func=GELU)
                h2g.append(h2)
                nc.vector.bn_stats(out=mv2[:, gg, 0:6], in_=h2[:])

        def emit_prep(g, h2g, s2, c2):
            """LN2 applies + T2 XBAR for all 4 quads of group g (consumed by
            emit_mm next iteration)."""
            z2Ts = []
            for q0 in range(0, G, B2):
                z2b = sb_c.tile([P, B2, D2], dt.bfloat16, name="z2b")
                for t in range(B2):
                    gg = q0 + t
                    nc.vector.tensor_scalar(
                        out=z2b[:, t, :], in0=h2g[gg][:],
                        scalar1=s2[:, gg:gg + 1], scalar2=c2[:, gg:gg + 1],
                        op0=A.mult, op1=A.subtract)
                z2T = sb_c.tile([P, 4 * B2, P], dt.bfloat16, name="z2T")
                nc.sync.dma_start(
                    z2T[:], z2b[:].rearrange("p a b -> p (a b)"),
                    transpose=True)
                z2Ts.append(z2T)
            return z2Ts

        def emit_mm(g, q0, z2T):
            """L2 matmuls, gelu2, x32, store for tiles [g*G+q0, +B2)."""
            h3s = outp.tile([P, B2, D3], dt.bfloat16, name="h3s")
            for t in range(B2):
                u2 = ps_b.tile([P, D3], dt.float32, name="u2", tag="psb2",
                               bufs=2)
                u2a, u2b = u2[:, 0:512], u2[:, 512:1024]
                for k in range(4):
                    nc.tensor.matmul(u2a[:], z2T[:, 4 * t + k, :],
                                     w2_sb[:, k, 0:512],
                                     start=(k == 0),
                                     stop=(k == 3 and not aug2))
                    nc.tensor.matmul(u2b[:], z2T[:, 4 * t + k, :],
                                     w2_sb[:, k, 512:1024],
                                     start=(k == 0),
                                     stop=(k == 3 and not aug2))
                if aug2:
                    nc.tensor.matmul(u2a[:], ones2[:], b2_sb[:, 0:512],
                                     start=False, stop=True)
                    nc.tensor.matmul(u2b[:], ones2[:], b2_sb[:, 512:1024],
                                     start=False, stop=True)
                nc.scalar.activation(out=h3s[:, t, :], in_=u2[:], func=GELU)
            flat = h3s[:].rearrange("p a b -> p (a b)")
            nc.vector.tensor_scalar(out=flat, in0=flat, scalar1=OUT_SCALE,
                                    scalar2=None, op0=A.mult)
            r0 = (g * G + q0) * P
            nc.gpsimd.dma_start(
                o_d[r0:r0 + B2 * P, :].rearrange("(a p) f -> p a f", p=P),
                h3s[:])

        load_x(0)
        if ngroups > 1:
            load_x(1)
        stage_A(0)
        stage_B(0)
        mmq = None
        for g in range(ngroups):
            if g + 2 < ngroups:
                load_x(g + 2)
            h1pk, s1, c1 = stage_L0(g)
            if mmq:
                emit_mm(g - 1, 0, mmq[0])
                emit_mm(g - 1, 4, mmq[1])
            h2g = []
            mv2 = stp.tile([P, G, 8], dt.float32, name="mv2")
            stage_C_quad(0, h1pk, s1, c1, h2g, mv2)
            stage_C_quad(4, h1pk, s1, c1, h2g, mv2)
            if mmq:
                emit_mm(g - 1, 8, mmq[2])
            stage_C_quad(8, h1pk, s1, c1, h2g, mv2)
            stage_C_quad(12, h1pk, s1, c1, h2g, mv2)
            s2, c2 = _ln_finish(nc, mybir, dt, stp, mv2, G, "2", 1.0 / D2)[:2]
            preps = emit_prep(g, h2g, s2, c2)
            if mmq:
                emit_mm(g - 1, 12, mmq[3])
            mmq = preps
            if g + 1 < ngroups:
                stage_A(g + 1)
                stage_B(g + 1)
        for q in range(4):
            emit_mm(ngroups - 1, 4 * q, mmq[q])
    return nc


def _prep_params(ln0_g, ln0_b, w0, b0, ln1_g, ln1_b, w1, b1, ln2_g, ln2_b,
                 w2, b2):
    """Fold LN affine into weights (fp64 on host). Returns DRAM arrays."""
    import ml_dtypes
    bf16 = ml_dtypes.bfloat16

    def fold(w, b, g, bl):
        wp = (w.astype(np.float64) * g.astype(np.float64)[None, :])
        bp = b.astype(np.float64) + wp @ bl.astype(np.float64)
        return wp, bp
    w0p, b0p = fold(w0, b0, ln0_g, ln0_b)
    w1p, b1p = fold(w1, b1, ln1_g, ln1_b)
    w2p, b2p = fold(w2, b2, ln2_g, ln2_b)
    aug0 = bool(np.any(b0p))
    # w0blk: [128, 512] block-diagonal: rows 32i..32i+5 x cols 128i..128(i+1)
    # hold w0'^T (+bias row at 32i+6 if aug0); zeros elsewhere kill the
    # garbage lanes of the packed transpose.
    w0blk = np.zeros((P, 4 * D1), dtype=bf16)
    for i in range(4):
        w0blk[32 * i:32 * i + F_IN, 128 * i:128 * (i + 1)] = \
            w0p.T.astype(bf16)
        if aug0:
            w0blk[32 * i + 6, 128 * i:128 * (i + 1)] = b0p.astype(bf16)
    w1t = np.ascontiguousarray(w1p.T.astype(bf16))
    w2t = np.ascontiguousarray(w2p.T.astype(bf16))
    b1aug = np.zeros((2, D2), dtype=bf16)
    b1aug[0] = b1p.astype(bf16)
    b2aug = np.zeros((2, D3), dtype=bf16)
    b2aug[0] = b2p.astype(bf16)
    aug1 = bool(np.any(b1p))
    aug2 = bool(np.any(b2p))
    return w0blk, w1t, w2t, b1aug, b2aug, aug0, aug1, aug2


def _get_compiled(rows, G, aug0, aug1, aug2, n_cores):
    key = (rows, G, aug0, aug1, aug2, n_cores)
    if key in _cache:
        return _cache[key]
    import concourse.tile as tile_mod
    from concourse import bacc
    nc = bacc.Bacc("TRN2", target_bir_lowering=False, debug=False,
                   num_devices=n_cores)
    _build(nc, tile_mod, rows, G, aug0, aug1, aug2)
    nc.compile()
    _cache[key] = nc
    return nc


def _prep_run(inputs):
    """Returns (compiled nc, per-core input maps) for the given full inputs."""
    w0blk, w1t, w2t, b1aug, b2aug, aug0, aug1, aug2 = _prep_params(
        *[np.asarray(inputs[k]) for k in
          ["ln0_g", "ln0_b", "w0", "b0", "ln1_g", "ln1_b",
           "w1", "b1", "ln2_g", "ln2_b", "w2", "b2"]])
    x = np.ascontiguousarray(np.asarray(inputs["x"]), dtype=np.float32)
    assert x.shape == (N_ROWS, F_IN)
    nc = _get_compiled(ROWS_PER_CORE, KERNEL_G, aug0, aug1, aug2, N_CORES)
    in_maps = []
    for c in range(N_CORES):
        in_maps.append({
            "x": x[c * ROWS_PER_CORE:(c + 1) * ROWS_PER_CORE],
            "w0blk": w0blk, "w1t": w1t, "w2t": w2t,
            "b1aug": b1aug, "b2aug": b2aug,
        })
    return nc, in_maps


def kernel(x, ln0_g, ln0_b, w0, b0, ln1_g, ln1_b, w1, b1, ln2_g, ln2_b,
           w2, b2):
    from concourse.bass_utils import run_bass_kernel_spmd
    nc, in_maps = _prep_run(dict(
        x=x, ln0_g=ln0_g, ln0_b=ln0_b, w0=w0, b0=b0, ln1_g=ln1_g,
        ln1_b=ln1_b, w1=w1, b1=b1, ln2_g=ln2_g, ln2_b=ln2_b, w2=w2, b2=b2))
    res = run_bass_kernel_spmd(nc, in_maps, core_ids=list(range(N_CORES)))
    return np.concatenate(
        [np.asarray(r["out"]) for r in res.results], axis=0
    ).astype(np.float32)
